# revision 8
# baseline (speedup 1.0000x reference)
"""Two-layer SAGEConv GNN on 8 Trainium2 NeuronCores — v5.

Strategy (graph/data parallel per sharding hint):
  - Nodes sharded across 8 cores (8750 rows each, padded to 9216).
    Within a core, nodes are sorted by in-degree DESCENDING and packed
    into 72 windows of 128; because the sequence is sorted, the max
    degree within a window is close to its mean, so the per-destination
    dense edge layout below pads only ~4%.
  - L1 computes BOTH first-layer projections (h = X@W1_l and
    xr = X@W1_r + b1, bias via a constant-1 input row) from a single X
    load, channel-major outputs, inputs split across both HWDGE queues
    and outputs drained through the GpSimd SWDGE queue.
  - Halo exchange at the launch boundary: the host gathers per-core h
    shards and builds a dense per-destination message table: block j of
    window w holds, at slot p (= the node's row within the window), the
    j-th in-edge's row h[src] * 1/deg[dst] (zero when deg < j).  The
    aggregation is then just acc^T[chan, row] += Msg_block^T @ I on
    TensorE — the one-hot scatter matrix is the IDENTITY by
    construction; no indirect DMA exists anywhere.
  - Degree-stratified precision: windows whose nodes all have deg >= 4
    (~97% of edges, quantization noise washed by the mean) ship their
    table blocks in float8_e3m4, scaled by a power of two into the
    normal range; the identity operand carries the exact 1/S
    compensation.  Low-degree tail windows stay bf16.
  - The self path is folded into the same PSUM accumulation via an
    identity-weight matmul; W2 is stationary as lhsT; in L3 the x2r +
    b2 terms ride one K=65 matmul ([I64; b2] against [x2r; ones]).

Three SPMD launches: L1 (projections), L2 (layer-1 aggregate + relu +
layer-2 projections), L3 (layer-2 aggregate + output).
"""
import numpy as np
import ml_dtypes

import concourse.bass as bass
import concourse.bacc as bacc
import concourse.mybir as mybir
import concourse.tile as tile
from concourse import bass_utils
from concourse.masks import make_identity

# ---------------------------------------------------------------- constants
N_NODES = 70000
N_EDGES = 500000
C_IN, C_HID, C_OUT = 1044, 128, 64
NCORES = 8
P = 128
SHARD = N_NODES // NCORES            # 8750
R = 9216                             # padded rows per core (multiple of 512)
NWIN = R // P                        # 72 windows per core
CT = 9                               # contraction tiles
KT = 117                             # rows per tile (9*117 = 1053 >= 1045)
CIN_P = CT * KT                      # 1053; row 1044 is the bias row
RSUP = 512                           # row super-block for L1
CHUNK_BLK = 48                       # min table blocks per DMA chunk
DEG_FP8 = 4                          # windows with min degree >= this: fp8
BF16 = mybir.dt.bfloat16
FP8 = mybir.dt.float8e3
F32 = mybir.dt.float32
E3M4 = ml_dtypes.float8_e3m4

_EXEC_NS = []                        # exec_time_ns per launch when profiling


# ------------------------------------------------------------- host helpers
def _bf16(x):
    return np.asarray(x, np.float32).astype(ml_dtypes.bfloat16)


def _prep_edges(src, dst):
    """Degree-sorted window layout with dense per-destination blocks.

    Edge at (block bstart[w]+j, slot p) on core m is the j-th in-edge of
    the node at window-row p; its table row is
    h_full[tabidx[...]] * tabscale[...] (zero rows where deg < j).
    """
    deg = np.bincount(dst, minlength=N_NODES).astype(np.int64)
    inv_deg = (1.0 / np.maximum(deg, 1.0)).astype(np.float32)

    perms = []
    pos_of = np.empty((NCORES, SHARD), np.int64)
    k_w = np.ones((NWIN,), np.int64)
    wsplit = NWIN
    for m in range(NCORES):
        d = deg[m * SHARD:(m + 1) * SHARD]
        order = np.argsort(-d, kind="stable")
        perm = np.full((R,), -1, np.int64)
        perm[:SHARD] = order
        pos_of[m, order] = np.arange(SHARD)
        perms.append(perm)
        dsrt = d[order]
        for w in range(NWIN):
            if w * P < SHARD:
                k_w[w] = max(k_w[w], dsrt[w * P])
        wsplit = min(wsplit, int((dsrt >= DEG_FP8).sum()) // P)
    bstart = np.concatenate(([0], np.cumsum(k_w)))
    B = int(k_w.sum())
    B8 = int(bstart[wsplit])

    core = dst // SHARD
    pos = pos_of[core, dst - core * SHARD]
    src_core = src // SHARD
    src_pad = src_core * R + pos_of[src_core, src - src_core * SHARD]

    tabidx_all, tabscale_all = [], []
    for m in range(NCORES):
        sel = np.nonzero(core == m)[0]
        order = np.argsort(pos[sel], kind="stable")
        sel = sel[order]
        p_sorted = pos[sel]
        # occurrence rank j within each destination's edge run
        starts = np.concatenate(([0], np.cumsum(np.bincount(
            p_sorted, minlength=R))))
        j = np.arange(len(sel)) - starts[p_sorted]
        win = p_sorted // P
        drel = p_sorted - win * P
        gslot = (bstart[win] + j) * P + drel

        tabidx = np.zeros((B * P,), np.int64)
        tabscale = np.zeros((B * P,), np.float32)
        tabidx[gslot] = src_pad[sel]
        tabscale[gslot] = inv_deg[dst[sel]]
        tabidx_all.append(tabidx)
        tabscale_all.append(tabscale)
    return k_w, bstart, B, B8, wsplit, tabidx_all, tabscale_all, perms


def _chunks(k_w, bstart, wlo, whi):
    """Split windows [wlo, whi) into chunks of >= CHUNK_BLK blocks."""
    out = []
    w0 = wlo
    while w0 < whi:
        w1 = w0 + 1
        while w1 < whi and bstart[w1 + 1] - bstart[w0] < CHUNK_BLK:
            w1 += 1
        out.append((w0, w1))
        w0 = w1
    return out


def _tab_rows(hcat_f32, tabidx, tabscale):
    return hcat_f32[tabidx] * tabscale[:, None]


def _pick_scale(rows_list, b8p):
    if b8p == 0:
        return 1.0
    mx = max(float(np.abs(r[:b8p]).max()) for r in rows_list)
    if mx <= 0:
        return 1.0
    s = 2.0 ** np.floor(np.log2(15.0 / mx))
    return float(min(max(s, 1.0 / 64.0), 64.0))


def _build_tab(rows, b8p, S, C):
    """(fp8-scaled, bf16) dense tables, partition = window row."""
    r8 = (rows[:b8p] * S).astype(E3M4)
    r16 = rows[b8p:].astype(ml_dtypes.bfloat16)

    def fold(r, dt):
        if r.shape[0] == 0:
            return np.zeros((P, C), dt)
        nb = r.shape[0] // P
        t = r.reshape(nb, P, C).transpose(1, 0, 2).reshape(P, nb * C)
        return np.ascontiguousarray(t)
    return fold(r8, E3M4), fold(r16, ml_dtypes.bfloat16)


# ------------------------------------------------------------ device builds
def _build_l1():
    nc = bacc.Bacc("TRN2", target_bir_lowering=False, debug=False,
                   num_devices=NCORES)
    nsup = R // RSUP
    xt = nc.dram_tensor("xt", [nsup * KT, CT * RSUP], BF16,
                        kind="ExternalInput")
    w1 = nc.dram_tensor("w1", [CIN_P, 2 * C_HID], BF16, kind="ExternalInput")
    ht_o = nc.dram_tensor("ht_o", [C_HID, R], BF16, kind="ExternalOutput")
    xrt_o = nc.dram_tensor("xrt_o", [C_HID, R], BF16, kind="ExternalOutput")
    half = CT * RSUP // 2

    with tile.TileContext(nc) as tc:
        with tc.tile_pool(name="cst", bufs=1) as cst, \
             tc.tile_pool(name="xp", bufs=3) as xp, \
             tc.tile_pool(name="ev", bufs=4) as ev, \
             tc.tile_pool(name="ph", bufs=2, space="PSUM") as ph, \
             tc.tile_pool(name="px", bufs=2, space="PSUM") as px:
            w1t = cst.tile([KT, CT * 2 * C_HID], BF16)
            for t in range(CT):
                eng = nc.sync if t % 2 == 0 else nc.scalar
                eng.dma_start(
                    out=w1t[:, t * 2 * C_HID:(t + 1) * 2 * C_HID],
                    in_=w1[t * KT:(t + 1) * KT, :])
            for rs in range(nsup):
                xtile = xp.tile([KT, CT * RSUP], BF16, tag="xtile")
                nc.sync.dma_start(
                    out=xtile[:, :half],
                    in_=xt[rs * KT:(rs + 1) * KT, :half])
                nc.scalar.dma_start(
                    out=xtile[:, half:],
                    in_=xt[rs * KT:(rs + 1) * KT, half:])
                acc_h = ph.tile([P, RSUP], F32, space="PSUM", tag="acc_h")
                acc_x = px.tile([P, RSUP], F32, space="PSUM", tag="acc_x")
                for t in range(CT):
                    nc.tensor.matmul(
                        out=acc_h[:],
                        lhsT=w1t[:, t * 2 * C_HID:t * 2 * C_HID + C_HID],
                        rhs=xtile[:, t * RSUP:(t + 1) * RSUP],
                        start=(t == 0), stop=(t == CT - 1))
                for t in range(CT):
                    nc.tensor.matmul(
                        out=acc_x[:],
                        lhsT=w1t[:, t * 2 * C_HID + C_HID:(t + 1) * 2 * C_HID],
                        rhs=xtile[:, t * RSUP:(t + 1) * RSUP],
                        start=(t == 0), stop=(t == CT - 1))
                hst = ev.tile([P, RSUP], BF16, tag="hst")
                nc.scalar.copy(out=hst[:], in_=acc_h[:])
                nc.gpsimd.dma_start(
                    out=ht_o[:, rs * RSUP:(rs + 1) * RSUP], in_=hst[:])
                xst = ev.tile([P, RSUP], BF16, tag="xst")
                nc.vector.tensor_copy(out=xst[:], in_=acc_x[:])
                nc.gpsimd.dma_start(
                    out=xrt_o[:, rs * RSUP:(rs + 1) * RSUP], in_=xst[:])
    nc.compile()
    return nc


def _build_l2(k_w, bstart, B, B8, wsplit):
    nc = bacc.Bacc("TRN2", target_bir_lowering=False, debug=False,
                   num_devices=NCORES)
    C = C_HID
    tab8 = nc.dram_tensor("tab8", [P, max(B8, 1) * C], FP8,
                          kind="ExternalInput")
    tab16 = nc.dram_tensor("tab16", [P, max(B - B8, 1) * C], BF16,
                           kind="ExternalInput")
    id8 = nc.dram_tensor("id8", [P, P], FP8, kind="ExternalInput")
    xrt = nc.dram_tensor("xrt", [C_HID, R], BF16, kind="ExternalInput")
    w2 = nc.dram_tensor("w2", [C_HID, P], BF16, kind="ExternalInput")
    o2 = nc.dram_tensor("o2", [P, R], BF16, kind="ExternalOutput")

    with tile.TileContext(nc) as tc:
        with tc.tile_pool(name="cst", bufs=1) as cst, \
             tc.tile_pool(name="tp", bufs=3) as tp, \
             tc.tile_pool(name="op", bufs=2) as op, \
             tc.tile_pool(name="ev", bufs=4) as ev, \
             tc.tile_pool(name="ps", bufs=3, space="PSUM") as ps, \
             tc.tile_pool(name="ps2", bufs=3, space="PSUM") as ps2:
            xrtt = cst.tile([P, R], BF16)
            nc.gpsimd.dma_start(out=xrtt[:], in_=xrt[:])
            w2t = cst.tile([P, P], BF16)
            nc.scalar.dma_start(out=w2t[:], in_=w2[:])
            id8t = cst.tile([P, P], FP8)
            nc.sync.dma_start(out=id8t[:], in_=id8[:])
            ident = cst.tile([P, P], BF16)
            make_identity(nc, ident[:])

            regions = [(0, wsplit, tab8, FP8, id8t, 0),
                       (wsplit, NWIN, tab16, BF16, ident, int(bstart[wsplit]))]
            ci = 0
            for (wlo, whi, tabsrc, tdt, rhsid, bb) in regions:
                for (w0, w1) in _chunks(k_w, bstart, wlo, whi):
                    cb0, cb1 = int(bstart[w0]) - bb, int(bstart[w1]) - bb
                    tabt = tp.tile([P, (cb1 - cb0) * C], tdt, tag="tabt")
                    hcol = (cb1 - cb0) * C // 2
                    eng0, eng1 = ((nc.sync, nc.scalar) if ci % 2 == 0
                                  else (nc.scalar, nc.sync))
                    ci += 1
                    eng0.dma_start(out=tabt[:, :hcol],
                                   in_=tabsrc[:, cb0 * C:cb0 * C + hcol])
                    eng1.dma_start(out=tabt[:, hcol:],
                                   in_=tabsrc[:, cb0 * C + hcol:cb1 * C])
                    o2c = op.tile([P, (w1 - w0) * P], BF16, tag="o2c")
                    for w in range(w0, w1):
                        boff = int(bstart[w]) - bb - cb0
                        kw = int(k_w[w])
                        acc = ps.tile([P, P], F32, space="PSUM", tag="acc")
                        for j in range(kw):
                            nc.tensor.matmul(
                                out=acc[:],
                                lhsT=tabt[:, (boff + j) * C:(boff + j + 1) * C],
                                rhs=rhsid[:],
                                start=(j == 0), stop=False)
                        nc.tensor.matmul(out=acc[:], lhsT=ident[:],
                                         rhs=xrtt[:, w * P:(w + 1) * P],
                                         start=False, stop=True)
                        x2t = ev.tile([P, P], BF16, tag="x2t")
                        nc.scalar.activation(
                            x2t[:], acc[:], mybir.ActivationFunctionType.Relu)
                        acc2 = ps2.tile([P, P], F32, space="PSUM", tag="acc2")
                        nc.tensor.matmul(out=acc2[:], lhsT=w2t[:], rhs=x2t[:],
                                         start=True, stop=True)
                        nc.scalar.copy(
                            out=o2c[:, (w - w0) * P:(w - w0 + 1) * P],
                            in_=acc2[:])
                    nc.gpsimd.dma_start(out=o2[:, w0 * P:w1 * P], in_=o2c[:])
    nc.compile()
    return nc


def _build_l3(k_w, bstart, B, B8, wsplit):
    nc = bacc.Bacc("TRN2", target_bir_lowering=False, debug=False,
                   num_devices=NCORES)
    C = C_OUT
    tab8 = nc.dram_tensor("tab8", [P, max(B8, 1) * C], FP8,
                          kind="ExternalInput")
    tab16 = nc.dram_tensor("tab16", [P, max(B - B8, 1) * C], BF16,
                           kind="ExternalInput")
    id8 = nc.dram_tensor("id8", [P, P], FP8, kind="ExternalInput")
    x2rt = nc.dram_tensor("x2rt", [C_OUT, R], BF16, kind="ExternalInput")
    b2 = nc.dram_tensor("b2", [1, C_OUT], BF16, kind="ExternalInput")
    out = nc.dram_tensor("out", [C_OUT, R], BF16, kind="ExternalOutput")

    with tile.TileContext(nc) as tc:
        with tc.tile_pool(name="cst", bufs=1) as cst, \
             tc.tile_pool(name="tp", bufs=3) as tp, \
             tc.tile_pool(name="op", bufs=2) as op, \
             tc.tile_pool(name="ps", bufs=4, space="PSUM") as ps:
            x2rtt = cst.tile([C_OUT + 1, R], BF16)
            nc.gpsimd.dma_start(out=x2rtt[:C_OUT, :], in_=x2rt[:])
            nc.vector.memset(x2rtt[C_OUT:C_OUT + 1, :], 1.0)
            ib2 = cst.tile([C_OUT + 1, C_OUT], BF16)
            make_identity(nc, ib2[:C_OUT, :])
            nc.scalar.dma_start(out=ib2[C_OUT:C_OUT + 1, :], in_=b2[:])
            id8t = cst.tile([P, P], FP8)
            nc.sync.dma_start(out=id8t[:], in_=id8[:])
            ident = cst.tile([P, P], BF16)
            make_identity(nc, ident[:])

            regions = [(0, wsplit, tab8, FP8, id8t, 0),
                       (wsplit, NWIN, tab16, BF16, ident, int(bstart[wsplit]))]
            ci = 0
            for (wlo, whi, tabsrc, tdt, rhsid, bb) in regions:
                for (w0, w1) in _chunks(k_w, bstart, wlo, whi):
                    cb0, cb1 = int(bstart[w0]) - bb, int(bstart[w1]) - bb
                    tabt = tp.tile([P, (cb1 - cb0) * C], tdt, tag="tabt")
                    hcol = (cb1 - cb0) * C // 2
                    eng0, eng1 = ((nc.sync, nc.scalar) if ci % 2 == 0
                                  else (nc.scalar, nc.sync))
                    ci += 1
                    eng0.dma_start(out=tabt[:, :hcol],
                                   in_=tabsrc[:, cb0 * C:cb0 * C + hcol])
                    eng1.dma_start(out=tabt[:, hcol:],
                                   in_=tabsrc[:, cb0 * C + hcol:cb1 * C])
                    outc = op.tile([C_OUT, (w1 - w0) * P], BF16, tag="outc")
                    for w in range(w0, w1):
                        boff = int(bstart[w]) - bb - cb0
                        kw = int(k_w[w])
                        acc = ps.tile([C_OUT, P], F32, space="PSUM", tag="acc")
                        for j in range(kw):
                            nc.tensor.matmul(
                                out=acc[:],
                                lhsT=tabt[:, (boff + j) * C:(boff + j + 1) * C],
                                rhs=rhsid[:],
                                start=(j == 0), stop=False)
                        nc.tensor.matmul(out=acc[:], lhsT=ib2[:],
                                         rhs=x2rtt[:, w * P:(w + 1) * P],
                                         start=False, stop=True)
                        nc.scalar.copy(
                            out=outc[:, (w - w0) * P:(w - w0 + 1) * P],
                            in_=acc[:])
                    nc.gpsimd.dma_start(out=out[:, w0 * P:w1 * P], in_=outc[:])
    nc.compile()
    return nc


# ------------------------------------------------------------------- driver
def _run(nc, in_maps, trace=False):
    res = bass_utils.run_bass_kernel_spmd(
        nc, in_maps, core_ids=list(range(NCORES)), trace=trace)
    if res.exec_time_ns:
        _EXEC_NS.append(res.exec_time_ns)
    return res.results


def kernel(features, edges, edges2, edge_features,
           W1_l, b1_l, W1_r, W2_l, b2_l, W2_r, _trace=False):
    features = np.asarray(features, np.float32)
    src = np.asarray(edges[0], np.int64)
    dst = np.asarray(edges[1], np.int64)
    _EXEC_NS.clear()

    # ---- host prep
    (k_w, bstart, B, B8, wsplit, tabidx_all, tabscale_all,
     perms) = _prep_edges(src, dst)

    w1cat = np.zeros((CIN_P, 2 * C_HID), np.float32)
    w1cat[:C_IN, :C_HID] = np.asarray(W1_l, np.float32)
    w1cat[:C_IN, C_HID:] = np.asarray(W1_r, np.float32)
    w1cat[C_IN, C_HID:] = np.asarray(b1_l, np.float32)  # constant-1 row
    w1cat = _bf16(w1cat)

    w2cat = _bf16(np.concatenate([np.asarray(W2_l, np.float32),
                                  np.asarray(W2_r, np.float32)], axis=1))
    b2row = _bf16(np.asarray(b2_l, np.float32).reshape(1, C_OUT))

    nsup = R // RSUP
    xts = []
    for m in range(NCORES):
        xt = np.zeros((CIN_P, R), ml_dtypes.bfloat16)
        perm = perms[m]
        cols = np.nonzero(perm >= 0)[0]
        xt[:C_IN, cols] = features[m * SHARD + perm[cols]].T
        xt[C_IN, cols] = 1.0
        xt3 = (xt.reshape(CT, KT, nsup, RSUP).transpose(2, 1, 0, 3)
               .reshape(nsup * KT, CT * RSUP))
        xts.append(np.ascontiguousarray(xt3))

    # ---- L1: both projections, channel-major
    nc1 = _build_l1()
    res1 = _run(nc1, [dict(xt=xts[m], w1=w1cat) for m in range(NCORES)],
                trace=_trace)
    hcat = np.concatenate(
        [np.asarray(res1[m]["ht_o"]).T for m in range(NCORES)],
        axis=0).astype(np.float32)

    # ---- L2: layer-1 aggregation + relu + layer-2 projections
    rows_all = [_tab_rows(hcat, tabidx_all[m], tabscale_all[m])
                for m in range(NCORES)]
    S2 = _pick_scale(rows_all, B8 * P)
    id8m = np.ascontiguousarray(
        (np.eye(P, dtype=np.float32) / S2).astype(E3M4))
    nc2 = _build_l2(k_w, bstart, B, B8, wsplit)
    in2 = []
    for m in range(NCORES):
        t8, t16 = _build_tab(rows_all[m], B8 * P, S2, C_HID)
        in2.append(dict(tab8=t8, tab16=t16, id8=id8m,
                        xrt=res1[m]["xrt_o"], w2=w2cat))
    res2 = _run(nc2, in2, trace=_trace)
    h2cat = np.concatenate(
        [np.asarray(res2[m]["o2"])[:C_OUT].T for m in range(NCORES)],
        axis=0).astype(np.float32)

    # ---- L3: layer-2 aggregation + output
    rows3_all = [_tab_rows(h2cat, tabidx_all[m], tabscale_all[m])
                 for m in range(NCORES)]
    S3 = _pick_scale(rows3_all, B8 * P)
    id83 = np.ascontiguousarray(
        (np.eye(P, dtype=np.float32) / S3).astype(E3M4))
    nc3 = _build_l3(k_w, bstart, B, B8, wsplit)
    in3 = []
    for m in range(NCORES):
        t8, t16 = _build_tab(rows3_all[m], B8 * P, S3, C_OUT)
        in3.append(dict(tab8=t8, tab16=t16, id8=id83,
                        x2rt=np.ascontiguousarray(
                            np.asarray(res2[m]["o2"])[C_OUT:]),
                        b2=b2row))
    res3 = _run(nc3, in3, trace=_trace)

    out = np.empty((N_NODES, C_OUT), np.float32)
    for m in range(NCORES):
        perm = perms[m]
        pos = np.nonzero(perm >= 0)[0]
        out[m * SHARD + perm[pos]] = (
            np.asarray(res3[m]["out"]).T.astype(np.float32)[pos])
    return np.ascontiguousarray(out)


# revision 9
# speedup vs baseline: 1.2083x; 1.2083x over previous
"""Two-layer SAGEConv GNN on 8 Trainium2 NeuronCores — v5.

Strategy (graph/data parallel per sharding hint):
  - Nodes sharded across 8 cores (8750 rows each, padded to 9216).
    Within a core, nodes are sorted by in-degree DESCENDING and packed
    into 72 windows of 128; because the sequence is sorted, the max
    degree within a window is close to its mean, so the per-destination
    dense edge layout below pads only ~4%.
  - L1 computes BOTH first-layer projections (h = X@W1_l and
    xr = X@W1_r + b1, bias via a constant-1 input row) from a single X
    load, channel-major outputs, inputs split across both HWDGE queues
    and outputs drained through the GpSimd SWDGE queue.
  - Halo exchange at the launch boundary: the host gathers per-core h
    shards and builds a dense per-destination message table: block j of
    window w holds, at slot p (= the node's row within the window), the
    j-th in-edge's row h[src] * 1/deg[dst] (zero when deg < j).  The
    aggregation is then just acc^T[chan, row] += Msg_block^T @ I on
    TensorE — the one-hot scatter matrix is the IDENTITY by
    construction; no indirect DMA exists anywhere.
  - Degree-stratified precision: windows whose nodes all have deg >= 4
    (~97% of edges, quantization noise washed by the mean) ship their
    table blocks in float8_e3m4, scaled by a power of two into the
    normal range; the identity operand carries the exact 1/S
    compensation.  Low-degree tail windows stay bf16.
  - The self path is folded into the same PSUM accumulation via an
    identity-weight matmul; W2 is stationary as lhsT; in L3 the x2r +
    b2 terms ride one K=65 matmul ([I64; b2] against [x2r; ones]).

Three SPMD launches: L1 (projections), L2 (layer-1 aggregate + relu +
layer-2 projections), L3 (layer-2 aggregate + output).
"""
import numpy as np
import ml_dtypes

import concourse.bass as bass
import concourse.bacc as bacc
import concourse.mybir as mybir
import concourse.tile as tile
from concourse import bass_utils
from concourse.masks import make_identity

# ---------------------------------------------------------------- constants
N_NODES = 70000
N_EDGES = 500000
C_IN, C_HID, C_OUT = 1044, 128, 64
NCORES = 8
P = 128
SHARD = N_NODES // NCORES            # 8750
R = 9216                             # padded rows per core (multiple of 512)
NWIN = R // P                        # 72 windows per core
CT = 9                               # contraction tiles
KT = 117                             # rows per tile (9*117 = 1053 >= 1045)
CIN_P = CT * KT                      # 1053; row 1044 is the bias row
RSUP = 512                           # row super-block for L1
CHUNK_BLK = 48                       # min table blocks per DMA chunk
DEG_FP8 = 4                          # windows with min degree >= this: fp8
BF16 = mybir.dt.bfloat16
FP8 = mybir.dt.float8e3
F32 = mybir.dt.float32
E3M4 = ml_dtypes.float8_e3m4

_EXEC_NS = []                        # exec_time_ns per launch when profiling


# ------------------------------------------------------------- host helpers
def _bf16(x):
    return np.asarray(x, np.float32).astype(ml_dtypes.bfloat16)


def _prep_edges(src, dst):
    """Degree-sorted window layout with dense per-destination blocks.

    Edge at (block bstart[w]+j, slot p) on core m is the j-th in-edge of
    the node at window-row p; its table row is
    h_full[tabidx[...]] * tabscale[...] (zero rows where deg < j).
    """
    deg = np.bincount(dst, minlength=N_NODES).astype(np.int64)
    inv_deg = (1.0 / np.maximum(deg, 1.0)).astype(np.float32)

    perms = []
    pos_of = np.empty((NCORES, SHARD), np.int64)
    k_w = np.ones((NWIN,), np.int64)
    wsplit = NWIN
    for m in range(NCORES):
        d = deg[m * SHARD:(m + 1) * SHARD]
        order = np.argsort(-d, kind="stable")
        perm = np.full((R,), -1, np.int64)
        perm[:SHARD] = order
        pos_of[m, order] = np.arange(SHARD)
        perms.append(perm)
        dsrt = d[order]
        for w in range(NWIN):
            if w * P < SHARD:
                k_w[w] = max(k_w[w], dsrt[w * P])
        wsplit = min(wsplit, int((dsrt >= DEG_FP8).sum()) // P)
    bstart = np.concatenate(([0], np.cumsum(k_w)))
    B = int(k_w.sum())
    B8 = int(bstart[wsplit])

    core = dst // SHARD
    pos = pos_of[core, dst - core * SHARD]
    src_core = src // SHARD
    src_pad = src_core * R + pos_of[src_core, src - src_core * SHARD]

    tabidx_all, tabscale_all = [], []
    for m in range(NCORES):
        sel = np.nonzero(core == m)[0]
        order = np.argsort(pos[sel], kind="stable")
        sel = sel[order]
        p_sorted = pos[sel]
        # occurrence rank j within each destination's edge run
        starts = np.concatenate(([0], np.cumsum(np.bincount(
            p_sorted, minlength=R))))
        j = np.arange(len(sel)) - starts[p_sorted]
        win = p_sorted // P
        drel = p_sorted - win * P
        gslot = (bstart[win] + j) * P + drel

        tabidx = np.zeros((B * P,), np.int64)
        tabscale = np.zeros((B * P,), np.float32)
        tabidx[gslot] = src_pad[sel]
        tabscale[gslot] = inv_deg[dst[sel]]
        tabidx_all.append(tabidx)
        tabscale_all.append(tabscale)
    return k_w, bstart, B, B8, wsplit, tabidx_all, tabscale_all, perms


def _chunks(k_w, bstart, wlo, whi):
    """Split windows [wlo, whi) into chunks of >= CHUNK_BLK blocks.

    The first chunk is kept small so the PE starts early; the rest are
    large to amortize DMA issues.
    """
    out = []
    w0 = wlo
    while w0 < whi:
        w1 = w0 + 1
        tgt = CHUNK_BLK // 4 if w0 == wlo else CHUNK_BLK
        while w1 < whi and bstart[w1 + 1] - bstart[w0] < tgt:
            w1 += 1
        out.append((w0, w1))
        w0 = w1
    return out


def _tab_rows(hcat_f32, tabidx, tabscale):
    return hcat_f32[tabidx] * tabscale[:, None]


def _pick_scale(rows_list, b8p):
    if b8p == 0:
        return 1.0
    mx = max(float(np.abs(r[:b8p]).max()) for r in rows_list)
    if mx <= 0:
        return 1.0
    s = 2.0 ** np.floor(np.log2(15.0 / mx))
    return float(min(max(s, 1.0 / 64.0), 64.0))


def _build_tab(rows, b8p, S, C):
    """(fp8-scaled, bf16) dense tables, partition = window row."""
    r8 = (rows[:b8p] * S).astype(E3M4)
    r16 = rows[b8p:].astype(ml_dtypes.bfloat16)

    def fold(r, dt):
        if r.shape[0] == 0:
            return np.zeros((P, C), dt)
        nb = r.shape[0] // P
        t = r.reshape(nb, P, C).transpose(1, 0, 2).reshape(P, nb * C)
        return np.ascontiguousarray(t)
    return fold(r8, E3M4), fold(r16, ml_dtypes.bfloat16)


# ------------------------------------------------------------ device builds
def _build_l1():
    nc = bacc.Bacc("TRN2", target_bir_lowering=False, debug=False,
                   num_devices=NCORES)
    nsup = R // RSUP
    xt = nc.dram_tensor("xt", [nsup * KT, CT * RSUP], BF16,
                        kind="ExternalInput")
    w1 = nc.dram_tensor("w1", [CIN_P, 2 * C_HID], BF16, kind="ExternalInput")
    ht_o = nc.dram_tensor("ht_o", [C_HID, R], BF16, kind="ExternalOutput")
    xrt_o = nc.dram_tensor("xrt_o", [C_HID, R], BF16, kind="ExternalOutput")
    half = CT * RSUP // 2

    with tile.TileContext(nc) as tc:
        with tc.tile_pool(name="cst", bufs=1) as cst, \
             tc.tile_pool(name="xp", bufs=3) as xp, \
             tc.tile_pool(name="ev", bufs=4) as ev, \
             tc.tile_pool(name="ph", bufs=2, space="PSUM") as ph, \
             tc.tile_pool(name="px", bufs=2, space="PSUM") as px:
            w1t = cst.tile([KT, CT * 2 * C_HID], BF16)
            for t in range(CT):
                eng = nc.sync if t % 2 == 0 else nc.scalar
                eng.dma_start(
                    out=w1t[:, t * 2 * C_HID:(t + 1) * 2 * C_HID],
                    in_=w1[t * KT:(t + 1) * KT, :])
            for rs in range(nsup):
                xtile = xp.tile([KT, CT * RSUP], BF16, tag="xtile")
                nc.sync.dma_start(
                    out=xtile[:, :half],
                    in_=xt[rs * KT:(rs + 1) * KT, :half])
                nc.scalar.dma_start(
                    out=xtile[:, half:],
                    in_=xt[rs * KT:(rs + 1) * KT, half:])
                acc_h = ph.tile([P, RSUP], F32, space="PSUM", tag="acc_h")
                acc_x = px.tile([P, RSUP], F32, space="PSUM", tag="acc_x")
                for t in range(CT):
                    nc.tensor.matmul(
                        out=acc_h[:],
                        lhsT=w1t[:, t * 2 * C_HID:t * 2 * C_HID + C_HID],
                        rhs=xtile[:, t * RSUP:(t + 1) * RSUP],
                        start=(t == 0), stop=(t == CT - 1))
                for t in range(CT):
                    nc.tensor.matmul(
                        out=acc_x[:],
                        lhsT=w1t[:, t * 2 * C_HID + C_HID:(t + 1) * 2 * C_HID],
                        rhs=xtile[:, t * RSUP:(t + 1) * RSUP],
                        start=(t == 0), stop=(t == CT - 1))
                hst = ev.tile([P, RSUP], BF16, tag="hst")
                nc.scalar.copy(out=hst[:], in_=acc_h[:])
                nc.gpsimd.dma_start(
                    out=ht_o[:, rs * RSUP:(rs + 1) * RSUP], in_=hst[:])
                xst = ev.tile([P, RSUP], BF16, tag="xst")
                nc.vector.tensor_copy(out=xst[:], in_=acc_x[:])
                weng = nc.sync if rs % 2 == 0 else nc.scalar
                weng.dma_start(
                    out=xrt_o[:, rs * RSUP:(rs + 1) * RSUP], in_=xst[:])
    nc.compile()
    return nc


def _build_l2(k_w, bstart, B, B8, wsplit):
    nc = bacc.Bacc("TRN2", target_bir_lowering=False, debug=False,
                   num_devices=NCORES)
    C = C_HID
    tab8 = nc.dram_tensor("tab8", [P, max(B8, 1) * C], FP8,
                          kind="ExternalInput")
    tab16 = nc.dram_tensor("tab16", [P, max(B - B8, 1) * C], BF16,
                           kind="ExternalInput")
    id8 = nc.dram_tensor("id8", [P, P], FP8, kind="ExternalInput")
    xrt = nc.dram_tensor("xrt", [C_HID, R], BF16, kind="ExternalInput")
    w2 = nc.dram_tensor("w2", [C_HID, P], BF16, kind="ExternalInput")
    o2 = nc.dram_tensor("o2", [P, R], BF16, kind="ExternalOutput")

    with tile.TileContext(nc) as tc:
        with tc.tile_pool(name="cst", bufs=1) as cst, \
             tc.tile_pool(name="tp", bufs=3) as tp, \
             tc.tile_pool(name="op", bufs=2) as op, \
             tc.tile_pool(name="ev", bufs=4) as ev, \
             tc.tile_pool(name="ps", bufs=4, space="PSUM") as ps, \
             tc.tile_pool(name="ps2", bufs=3, space="PSUM") as ps2:
            xrtt = cst.tile([P, R], BF16)
            nc.gpsimd.dma_start(out=xrtt[:], in_=xrt[:])
            w2t = cst.tile([P, P], BF16)
            nc.scalar.dma_start(out=w2t[:], in_=w2[:])
            id8t = cst.tile([P, P], FP8)
            nc.sync.dma_start(out=id8t[:], in_=id8[:])
            ident = cst.tile([P, P], BF16)
            make_identity(nc, ident[:])

            regions = [(0, wsplit, tab8, FP8, id8t, 0),
                       (wsplit, NWIN, tab16, BF16, ident, int(bstart[wsplit]))]
            ci = 0
            for (wlo, whi, tabsrc, tdt, rhsid, bb) in regions:
                for (w0, w1) in _chunks(k_w, bstart, wlo, whi):
                    cb0, cb1 = int(bstart[w0]) - bb, int(bstart[w1]) - bb
                    tabt = tp.tile([P, (cb1 - cb0) * C], tdt, tag="tabt")
                    hcol = (cb1 - cb0) * C // 2
                    eng0, eng1 = ((nc.sync, nc.scalar) if ci % 2 == 0
                                  else (nc.scalar, nc.sync))
                    ci += 1
                    eng0.dma_start(out=tabt[:, :hcol],
                                   in_=tabsrc[:, cb0 * C:cb0 * C + hcol])
                    eng1.dma_start(out=tabt[:, hcol:],
                                   in_=tabsrc[:, cb0 * C + hcol:cb1 * C])
                    o2c = op.tile([P, (w1 - w0) * P], BF16, tag="o2c")
                    for w in range(w0, w1):
                        boff = int(bstart[w]) - bb - cb0
                        kw = int(k_w[w])
                        acc = ps.tile([P, P], F32, space="PSUM", tag="acc")
                        for j in range(kw):
                            nc.tensor.matmul(
                                out=acc[:],
                                lhsT=tabt[:, (boff + j) * C:(boff + j + 1) * C],
                                rhs=rhsid[:],
                                start=(j == 0), stop=False)
                        nc.tensor.matmul(out=acc[:], lhsT=ident[:],
                                         rhs=xrtt[:, w * P:(w + 1) * P],
                                         start=False, stop=True)
                        x2t = ev.tile([P, P], BF16, tag="x2t")
                        nc.vector.tensor_scalar_max(x2t[:], acc[:], 0.0)
                        acc2 = ps2.tile([P, P], F32, space="PSUM", tag="acc2")
                        nc.tensor.matmul(out=acc2[:], lhsT=w2t[:], rhs=x2t[:],
                                         start=True, stop=True)
                        nc.scalar.copy(
                            out=o2c[:, (w - w0) * P:(w - w0 + 1) * P],
                            in_=acc2[:])
                    nc.gpsimd.dma_start(out=o2[:, w0 * P:w1 * P], in_=o2c[:])
    nc.compile()
    return nc


def _build_l3(k_w, bstart, B, B8, wsplit):
    nc = bacc.Bacc("TRN2", target_bir_lowering=False, debug=False,
                   num_devices=NCORES)
    C = C_OUT
    tab8 = nc.dram_tensor("tab8", [P, max(B8, 1) * C], FP8,
                          kind="ExternalInput")
    tab16 = nc.dram_tensor("tab16", [P, max(B - B8, 1) * C], BF16,
                           kind="ExternalInput")
    id8 = nc.dram_tensor("id8", [P, P], FP8, kind="ExternalInput")
    x2rt = nc.dram_tensor("x2rt", [C_OUT, R], BF16, kind="ExternalInput")
    b2 = nc.dram_tensor("b2", [1, C_OUT], BF16, kind="ExternalInput")
    out = nc.dram_tensor("out", [C_OUT, R], BF16, kind="ExternalOutput")

    with tile.TileContext(nc) as tc:
        with tc.tile_pool(name="cst", bufs=1) as cst, \
             tc.tile_pool(name="tp", bufs=3) as tp, \
             tc.tile_pool(name="op", bufs=2) as op, \
             tc.tile_pool(name="ps", bufs=4, space="PSUM") as ps:
            x2rtt = cst.tile([C_OUT + 1, R], BF16)
            nc.gpsimd.dma_start(out=x2rtt[:C_OUT, :], in_=x2rt[:])
            nc.vector.memset(x2rtt[C_OUT:C_OUT + 1, :], 1.0)
            ib2 = cst.tile([C_OUT + 1, C_OUT], BF16)
            make_identity(nc, ib2[:C_OUT, :])
            nc.scalar.dma_start(out=ib2[C_OUT:C_OUT + 1, :], in_=b2[:])
            id8t = cst.tile([P, P], FP8)
            nc.sync.dma_start(out=id8t[:], in_=id8[:])
            ident = cst.tile([P, P], BF16)
            make_identity(nc, ident[:])

            regions = [(0, wsplit, tab8, FP8, id8t, 0),
                       (wsplit, NWIN, tab16, BF16, ident, int(bstart[wsplit]))]
            ci = 0
            for (wlo, whi, tabsrc, tdt, rhsid, bb) in regions:
                for (w0, w1) in _chunks(k_w, bstart, wlo, whi):
                    cb0, cb1 = int(bstart[w0]) - bb, int(bstart[w1]) - bb
                    tabt = tp.tile([P, (cb1 - cb0) * C], tdt, tag="tabt")
                    hcol = (cb1 - cb0) * C // 2
                    eng0, eng1 = ((nc.sync, nc.scalar) if ci % 2 == 0
                                  else (nc.scalar, nc.sync))
                    ci += 1
                    eng0.dma_start(out=tabt[:, :hcol],
                                   in_=tabsrc[:, cb0 * C:cb0 * C + hcol])
                    eng1.dma_start(out=tabt[:, hcol:],
                                   in_=tabsrc[:, cb0 * C + hcol:cb1 * C])
                    outc = op.tile([C_OUT, (w1 - w0) * P], BF16, tag="outc")
                    for w in range(w0, w1):
                        boff = int(bstart[w]) - bb - cb0
                        kw = int(k_w[w])
                        acc = ps.tile([C_OUT, P], F32, space="PSUM", tag="acc")
                        for j in range(kw):
                            nc.tensor.matmul(
                                out=acc[:],
                                lhsT=tabt[:, (boff + j) * C:(boff + j + 1) * C],
                                rhs=rhsid[:],
                                start=(j == 0), stop=False)
                        nc.tensor.matmul(out=acc[:], lhsT=ib2[:],
                                         rhs=x2rtt[:, w * P:(w + 1) * P],
                                         start=False, stop=True)
                        nc.scalar.copy(
                            out=outc[:, (w - w0) * P:(w - w0 + 1) * P],
                            in_=acc[:])
                    nc.gpsimd.dma_start(out=out[:, w0 * P:w1 * P], in_=outc[:])
    nc.compile()
    return nc


# ------------------------------------------------------------------- driver
def _run(nc, in_maps, trace=False):
    res = bass_utils.run_bass_kernel_spmd(
        nc, in_maps, core_ids=list(range(NCORES)), trace=trace)
    if res.exec_time_ns:
        _EXEC_NS.append(res.exec_time_ns)
    return res.results


def kernel(features, edges, edges2, edge_features,
           W1_l, b1_l, W1_r, W2_l, b2_l, W2_r, _trace=False):
    features = np.asarray(features, np.float32)
    src = np.asarray(edges[0], np.int64)
    dst = np.asarray(edges[1], np.int64)
    _EXEC_NS.clear()

    # ---- host prep
    (k_w, bstart, B, B8, wsplit, tabidx_all, tabscale_all,
     perms) = _prep_edges(src, dst)

    w1cat = np.zeros((CIN_P, 2 * C_HID), np.float32)
    w1cat[:C_IN, :C_HID] = np.asarray(W1_l, np.float32)
    w1cat[:C_IN, C_HID:] = np.asarray(W1_r, np.float32)
    w1cat[C_IN, C_HID:] = np.asarray(b1_l, np.float32)  # constant-1 row
    w1cat = _bf16(w1cat)

    w2cat = _bf16(np.concatenate([np.asarray(W2_l, np.float32),
                                  np.asarray(W2_r, np.float32)], axis=1))
    b2row = _bf16(np.asarray(b2_l, np.float32).reshape(1, C_OUT))

    nsup = R // RSUP
    xts = []
    for m in range(NCORES):
        xt = np.zeros((CIN_P, R), ml_dtypes.bfloat16)
        perm = perms[m]
        cols = np.nonzero(perm >= 0)[0]
        xt[:C_IN, cols] = features[m * SHARD + perm[cols]].T
        xt[C_IN, cols] = 1.0
        xt3 = (xt.reshape(CT, KT, nsup, RSUP).transpose(2, 1, 0, 3)
               .reshape(nsup * KT, CT * RSUP))
        xts.append(np.ascontiguousarray(xt3))

    # ---- L1: both projections, channel-major
    nc1 = _build_l1()
    res1 = _run(nc1, [dict(xt=xts[m], w1=w1cat) for m in range(NCORES)],
                trace=_trace)
    hcat = np.concatenate(
        [np.asarray(res1[m]["ht_o"]).T for m in range(NCORES)],
        axis=0).astype(np.float32)

    # ---- L2: layer-1 aggregation + relu + layer-2 projections
    rows_all = [_tab_rows(hcat, tabidx_all[m], tabscale_all[m])
                for m in range(NCORES)]
    S2 = _pick_scale(rows_all, B8 * P)
    id8m = np.ascontiguousarray(
        (np.eye(P, dtype=np.float32) / S2).astype(E3M4))
    nc2 = _build_l2(k_w, bstart, B, B8, wsplit)
    in2 = []
    for m in range(NCORES):
        t8, t16 = _build_tab(rows_all[m], B8 * P, S2, C_HID)
        in2.append(dict(tab8=t8, tab16=t16, id8=id8m,
                        xrt=res1[m]["xrt_o"], w2=w2cat))
    res2 = _run(nc2, in2, trace=_trace)
    h2cat = np.concatenate(
        [np.asarray(res2[m]["o2"])[:C_OUT].T for m in range(NCORES)],
        axis=0).astype(np.float32)

    # ---- L3: layer-2 aggregation + output
    rows3_all = [_tab_rows(h2cat, tabidx_all[m], tabscale_all[m])
                 for m in range(NCORES)]
    S3 = _pick_scale(rows3_all, B8 * P)
    id83 = np.ascontiguousarray(
        (np.eye(P, dtype=np.float32) / S3).astype(E3M4))
    nc3 = _build_l3(k_w, bstart, B, B8, wsplit)
    in3 = []
    for m in range(NCORES):
        t8, t16 = _build_tab(rows3_all[m], B8 * P, S3, C_OUT)
        in3.append(dict(tab8=t8, tab16=t16, id8=id83,
                        x2rt=np.ascontiguousarray(
                            np.asarray(res2[m]["o2"])[C_OUT:]),
                        b2=b2row))
    res3 = _run(nc3, in3, trace=_trace)

    out = np.empty((N_NODES, C_OUT), np.float32)
    for m in range(NCORES):
        perm = perms[m]
        pos = np.nonzero(perm >= 0)[0]
        out[m * SHARD + perm[pos]] = (
            np.asarray(res3[m]["out"]).T.astype(np.float32)[pos])
    return np.ascontiguousarray(out)


# revision 11
# speedup vs baseline: 1.3204x; 1.0927x over previous
"""Two-layer SAGEConv GNN on 8 Trainium2 NeuronCores — v5.

Strategy (graph/data parallel per sharding hint):
  - Nodes sharded across 8 cores (8750 rows each, padded to 9216).
    Within a core, nodes are sorted by in-degree DESCENDING and packed
    into 72 windows of 128; because the sequence is sorted, the max
    degree within a window is close to its mean, so the per-destination
    dense edge layout below pads only ~4%.
  - L1 computes BOTH first-layer projections (h = X@W1_l and
    xr = X@W1_r + b1, bias via a constant-1 input row) from a single X
    load, channel-major outputs, inputs split across both HWDGE queues
    and outputs drained through the GpSimd SWDGE queue.
  - Halo exchange at the launch boundary: the host gathers per-core h
    shards and builds a dense per-destination message table: block j of
    window w holds, at slot p (= the node's row within the window), the
    j-th in-edge's row h[src] * 1/deg[dst] (zero when deg < j).  The
    aggregation is then just acc^T[chan, row] += Msg_block^T @ I on
    TensorE — the one-hot scatter matrix is the IDENTITY by
    construction; no indirect DMA exists anywhere.
  - Degree-stratified precision: windows whose nodes all have deg >= 4
    (~97% of edges, quantization noise washed by the mean) ship their
    table blocks in float8_e3m4, scaled by a power of two into the
    normal range; the identity operand carries the exact 1/S
    compensation.  Low-degree tail windows stay bf16.
  - The self path is folded into the same PSUM accumulation via an
    identity-weight matmul; W2 is stationary as lhsT; in L3 the x2r +
    b2 terms ride one K=65 matmul ([I64; b2] against [x2r; ones]).

Three SPMD launches: L1 (projections), L2 (layer-1 aggregate + relu +
layer-2 projections), L3 (layer-2 aggregate + output).
"""
import numpy as np
import ml_dtypes

import concourse.bass as bass
import concourse.bacc as bacc
import concourse.mybir as mybir
import concourse.tile as tile
from concourse import bass_utils
from concourse.masks import make_identity

# ---------------------------------------------------------------- constants
N_NODES = 70000
N_EDGES = 500000
C_IN, C_HID, C_OUT = 1044, 128, 64
NCORES = 8
P = 128
SHARD = N_NODES // NCORES            # 8750
R = 9216                             # padded rows per core (multiple of 512)
NWIN = R // P                        # 72 windows per core
CT = 9                               # contraction tiles
KT = 117                             # rows per tile (9*117 = 1053 >= 1045)
CIN_P = CT * KT                      # 1053; row 1044 is the bias row
RSUP = 512                           # row super-block for L1
CHUNK_BLK = 64                       # min table blocks per DMA chunk
DEG_FP8 = 4                          # windows with min degree >= this: fp8
BF16 = mybir.dt.bfloat16
FP8 = mybir.dt.float8e3
F32 = mybir.dt.float32
E3M4 = ml_dtypes.float8_e3m4

_EXEC_NS = []                        # exec_time_ns per launch when profiling


# ------------------------------------------------------------- host helpers
def _bf16(x):
    return np.asarray(x, np.float32).astype(ml_dtypes.bfloat16)


def _prep_edges(src, dst):
    """Degree-sorted window layout with dense per-destination blocks.

    Edge at (block bstart[w]+j, slot p) on core m is the j-th in-edge of
    the node at window-row p; its table row is
    h_full[tabidx[...]] * tabscale[...] (zero rows where deg < j).
    """
    deg = np.bincount(dst, minlength=N_NODES).astype(np.int64)
    inv_deg = (1.0 / np.maximum(deg, 1.0)).astype(np.float32)

    perms = []
    pos_of = np.empty((NCORES, SHARD), np.int64)
    k_w = np.ones((NWIN,), np.int64)
    wsplit = NWIN
    for m in range(NCORES):
        d = deg[m * SHARD:(m + 1) * SHARD]
        order = np.argsort(-d, kind="stable")
        perm = np.full((R,), -1, np.int64)
        perm[:SHARD] = order
        pos_of[m, order] = np.arange(SHARD)
        perms.append(perm)
        dsrt = d[order]
        for w in range(NWIN):
            if w * P < SHARD:
                k_w[w] = max(k_w[w], dsrt[w * P])
        wsplit = min(wsplit, int((dsrt >= DEG_FP8).sum()) // P)
    bstart = np.concatenate(([0], np.cumsum(k_w)))
    B = int(k_w.sum())
    B8 = int(bstart[wsplit])

    core = dst // SHARD
    pos = pos_of[core, dst - core * SHARD]
    src_core = src // SHARD
    src_pad = src_core * R + pos_of[src_core, src - src_core * SHARD]

    tabidx_all, tabscale_all = [], []
    for m in range(NCORES):
        sel = np.nonzero(core == m)[0]
        order = np.argsort(pos[sel], kind="stable")
        sel = sel[order]
        p_sorted = pos[sel]
        # occurrence rank j within each destination's edge run
        starts = np.concatenate(([0], np.cumsum(np.bincount(
            p_sorted, minlength=R))))
        j = np.arange(len(sel)) - starts[p_sorted]
        win = p_sorted // P
        drel = p_sorted - win * P
        gslot = (bstart[win] + j) * P + drel

        tabidx = np.zeros((B * P,), np.int64)
        tabscale = np.zeros((B * P,), np.float32)
        tabidx[gslot] = src_pad[sel]
        tabscale[gslot] = inv_deg[dst[sel]]
        tabidx_all.append(tabidx)
        tabscale_all.append(tabscale)
    # window pairs for L3 (two 64-chan windows share one 128-col block)
    npair = NWIN // 2
    kp = np.maximum(k_w[0::2], k_w[1::2])
    pstart = np.concatenate(([0], np.cumsum(kp)))
    BPp = int(kp.sum())
    ps8 = wsplit // 2
    pl = np.full((BPp * P,), -1, np.int64)
    pr = np.full((BPp * P,), -1, np.int64)
    ar = np.arange(P)
    for p in range(npair):
        for j in range(int(kp[p])):
            base = (int(pstart[p]) + j) * P
            if j < k_w[2 * p]:
                pl[base:base + P] = (int(bstart[2 * p]) + j) * P + ar
            if j < k_w[2 * p + 1]:
                pr[base:base + P] = (int(bstart[2 * p + 1]) + j) * P + ar
    pairs = dict(kp=kp, pstart=pstart, BP=BPp, PS=ps8, pl=pl, pr=pr)
    return (k_w, bstart, B, B8, wsplit, tabidx_all, tabscale_all, perms,
            pairs)


def _chunks(k_w, bstart, wlo, whi):
    """Split windows [wlo, whi) into chunks of >= CHUNK_BLK blocks.

    The first chunk is kept small so the PE starts early; the rest are
    large to amortize DMA issues.
    """
    out = []
    w0 = wlo
    while w0 < whi:
        w1 = w0 + 1
        tgt = CHUNK_BLK // 4 if w0 == wlo else CHUNK_BLK
        while w1 < whi and bstart[w1 + 1] - bstart[w0] < tgt:
            w1 += 1
        out.append((w0, w1))
        w0 = w1
    return out


def _tab_rows(hcat_f32, tabidx, tabscale):
    return hcat_f32[tabidx] * tabscale[:, None]


def _pick_scale(rows_list, b8p):
    if b8p == 0:
        return 1.0
    mx = max(float(np.abs(r[:b8p]).max()) for r in rows_list)
    if mx <= 0:
        return 1.0
    s = 2.0 ** np.floor(np.log2(15.0 / mx))
    return float(min(max(s, 1.0 / 64.0), 64.0))


def _build_tab(rows, b8p, S, C):
    """(fp8-scaled, bf16) dense tables, partition = window row."""
    r8 = (rows[:b8p] * S).astype(E3M4)
    r16 = rows[b8p:].astype(ml_dtypes.bfloat16)

    def fold(r, dt):
        if r.shape[0] == 0:
            return np.zeros((P, C), dt)
        nb = r.shape[0] // P
        t = r.reshape(nb, P, C).transpose(1, 0, 2).reshape(P, nb * C)
        return np.ascontiguousarray(t)
    return fold(r8, E3M4), fold(r16, ml_dtypes.bfloat16)


def _pair_rows(rows3, pairs):
    """[BP*P, 128] rows: cols 0-63 = even window, 64-127 = odd window."""
    BPp = pairs["BP"]
    out = np.zeros((BPp * P, 2 * C_OUT), np.float32)
    ok = pairs["pl"] >= 0
    out[ok, :C_OUT] = rows3[pairs["pl"][ok]]
    ok = pairs["pr"] >= 0
    out[ok, C_OUT:] = rows3[pairs["pr"][ok]]
    return out


# ------------------------------------------------------------ device builds
def _build_l1():
    nc = bacc.Bacc("TRN2", target_bir_lowering=False, debug=False,
                   num_devices=NCORES)
    nsup = R // RSUP
    xt = nc.dram_tensor("xt", [nsup * KT, CT * RSUP], BF16,
                        kind="ExternalInput")
    w1 = nc.dram_tensor("w1", [CIN_P, 2 * C_HID], BF16, kind="ExternalInput")
    ht_o = nc.dram_tensor("ht_o", [C_HID, R], BF16, kind="ExternalOutput")
    xrt_o = nc.dram_tensor("xrt_o", [C_HID, R], BF16, kind="ExternalOutput")
    half = CT * RSUP // 2

    with tile.TileContext(nc) as tc:
        with tc.tile_pool(name="cst", bufs=1) as cst, \
             tc.tile_pool(name="xp", bufs=3) as xp, \
             tc.tile_pool(name="ev", bufs=4) as ev, \
             tc.tile_pool(name="ph", bufs=2, space="PSUM") as ph, \
             tc.tile_pool(name="px", bufs=2, space="PSUM") as px:
            w1t = cst.tile([KT, CT * 2 * C_HID], BF16)
            for t in range(CT):
                eng = nc.sync if t % 2 == 0 else nc.scalar
                eng.dma_start(
                    out=w1t[:, t * 2 * C_HID:(t + 1) * 2 * C_HID],
                    in_=w1[t * KT:(t + 1) * KT, :])
            for rs in range(nsup):
                xtile = xp.tile([KT, CT * RSUP], BF16, tag="xtile")
                nc.sync.dma_start(
                    out=xtile[:, :half],
                    in_=xt[rs * KT:(rs + 1) * KT, :half])
                nc.scalar.dma_start(
                    out=xtile[:, half:],
                    in_=xt[rs * KT:(rs + 1) * KT, half:])
                acc_h = ph.tile([P, RSUP], F32, space="PSUM", tag="acc_h")
                acc_x = px.tile([P, RSUP], F32, space="PSUM", tag="acc_x")
                for t in range(CT):
                    nc.tensor.matmul(
                        out=acc_h[:],
                        lhsT=w1t[:, t * 2 * C_HID:t * 2 * C_HID + C_HID],
                        rhs=xtile[:, t * RSUP:(t + 1) * RSUP],
                        start=(t == 0), stop=(t == CT - 1))
                for t in range(CT):
                    nc.tensor.matmul(
                        out=acc_x[:],
                        lhsT=w1t[:, t * 2 * C_HID + C_HID:(t + 1) * 2 * C_HID],
                        rhs=xtile[:, t * RSUP:(t + 1) * RSUP],
                        start=(t == 0), stop=(t == CT - 1))
                hst = ev.tile([P, RSUP], BF16, tag="hst")
                nc.scalar.copy(out=hst[:], in_=acc_h[:])
                nc.gpsimd.dma_start(
                    out=ht_o[:, rs * RSUP:(rs + 1) * RSUP], in_=hst[:])
                xst = ev.tile([P, RSUP], BF16, tag="xst")
                nc.vector.tensor_copy(out=xst[:], in_=acc_x[:])
                weng = nc.sync if rs % 2 == 0 else nc.scalar
                weng.dma_start(
                    out=xrt_o[:, rs * RSUP:(rs + 1) * RSUP], in_=xst[:])
    nc.compile()
    return nc


def _build_l2(k_w, bstart, B, B8, wsplit):
    nc = bacc.Bacc("TRN2", target_bir_lowering=False, debug=False,
                   num_devices=NCORES)
    C = C_HID
    tab8 = nc.dram_tensor("tab8", [P, max(B8, 1) * C], FP8,
                          kind="ExternalInput")
    tab16 = nc.dram_tensor("tab16", [P, max(B - B8, 1) * C], BF16,
                           kind="ExternalInput")
    id8 = nc.dram_tensor("id8", [P, P], FP8, kind="ExternalInput")
    xrt = nc.dram_tensor("xrt", [C_HID, R], BF16, kind="ExternalInput")
    w2 = nc.dram_tensor("w2", [C_HID, P], BF16, kind="ExternalInput")
    o2 = nc.dram_tensor("o2", [P, R], BF16, kind="ExternalOutput")

    with tile.TileContext(nc) as tc:
        with tc.tile_pool(name="cst", bufs=1) as cst, \
             tc.tile_pool(name="tp", bufs=3) as tp, \
             tc.tile_pool(name="op", bufs=2) as op, \
             tc.tile_pool(name="ev", bufs=4) as ev, \
             tc.tile_pool(name="ps", bufs=4, space="PSUM") as ps, \
             tc.tile_pool(name="ps2", bufs=3, space="PSUM") as ps2:
            xrtt = cst.tile([P, R], BF16)
            nc.gpsimd.dma_start(out=xrtt[:], in_=xrt[:])
            w2t = cst.tile([P, P], BF16)
            nc.scalar.dma_start(out=w2t[:], in_=w2[:])
            id8t = cst.tile([P, P], FP8)
            nc.sync.dma_start(out=id8t[:], in_=id8[:])
            ident = cst.tile([P, P], BF16)
            make_identity(nc, ident[:])

            regions = [(0, wsplit, tab8, FP8, id8t, 0),
                       (wsplit, NWIN, tab16, BF16, ident, int(bstart[wsplit]))]
            ci = 0
            for (wlo, whi, tabsrc, tdt, rhsid, bb) in regions:
                for (w0, w1) in _chunks(k_w, bstart, wlo, whi):
                    cb0, cb1 = int(bstart[w0]) - bb, int(bstart[w1]) - bb
                    tabt = tp.tile([P, (cb1 - cb0) * C], tdt, tag="tabt")
                    hcol = (cb1 - cb0) * C // 2
                    eng0, eng1 = ((nc.sync, nc.scalar) if ci % 2 == 0
                                  else (nc.scalar, nc.sync))
                    ci += 1
                    eng0.dma_start(out=tabt[:, :hcol],
                                   in_=tabsrc[:, cb0 * C:cb0 * C + hcol])
                    eng1.dma_start(out=tabt[:, hcol:],
                                   in_=tabsrc[:, cb0 * C + hcol:cb1 * C])
                    o2c = op.tile([P, (w1 - w0) * P], BF16, tag="o2c")
                    for w in range(w0, w1):
                        boff = int(bstart[w]) - bb - cb0
                        kw = int(k_w[w])
                        acc = ps.tile([P, P], F32, space="PSUM", tag="acc")
                        for j in range(kw):
                            nc.tensor.matmul(
                                out=acc[:],
                                lhsT=tabt[:, (boff + j) * C:(boff + j + 1) * C],
                                rhs=rhsid[:],
                                start=(j == 0), stop=False)
                        nc.tensor.matmul(out=acc[:], lhsT=ident[:],
                                         rhs=xrtt[:, w * P:(w + 1) * P],
                                         start=False, stop=True)
                        x2t = ev.tile([P, P], BF16, tag="x2t")
                        nc.vector.tensor_scalar_max(x2t[:], acc[:], 0.0)
                        acc2 = ps2.tile([P, P], F32, space="PSUM", tag="acc2")
                        nc.tensor.matmul(out=acc2[:], lhsT=w2t[:], rhs=x2t[:],
                                         start=True, stop=True)
                        nc.scalar.copy(
                            out=o2c[:, (w - w0) * P:(w - w0 + 1) * P],
                            in_=acc2[:])
                    nc.gpsimd.dma_start(out=o2[:, w0 * P:w1 * P], in_=o2c[:])
    nc.compile()
    return nc


def _build_l3(pairs):
    nc = bacc.Bacc("TRN2", target_bir_lowering=False, debug=False,
                   num_devices=NCORES)
    kp, pstart = pairs["kp"], pairs["pstart"]
    BPp, PS = pairs["BP"], pairs["PS"]
    npair = NWIN // 2
    B8 = int(pstart[PS])
    tab8 = nc.dram_tensor("tab8", [P, max(B8, 1) * P], FP8,
                          kind="ExternalInput")
    tab16 = nc.dram_tensor("tab16", [P, max(BPp - B8, 1) * P], BF16,
                           kind="ExternalInput")
    id8 = nc.dram_tensor("id8", [P, P], FP8, kind="ExternalInput")
    x2rp = nc.dram_tensor("x2rp", [P, npair * P], BF16, kind="ExternalInput")
    b2c = nc.dram_tensor("b2c", [1, P], BF16, kind="ExternalInput")
    out = nc.dram_tensor("out", [P, npair * P], BF16, kind="ExternalOutput")

    with tile.TileContext(nc) as tc:
        with tc.tile_pool(name="cst", bufs=1) as cst, \
             tc.tile_pool(name="tp", bufs=3) as tp, \
             tc.tile_pool(name="op", bufs=2) as op, \
             tc.tile_pool(name="ps", bufs=4, space="PSUM") as ps:
            x2rpt = cst.tile([P, npair * P], BF16)
            nc.gpsimd.dma_start(out=x2rpt[:], in_=x2rp[:])
            b2ct = cst.tile([1, P], BF16)
            nc.scalar.dma_start(out=b2ct[:], in_=b2c[:])
            onest = cst.tile([1, P], BF16)
            nc.vector.memset(onest[:], 1.0)
            id8t = cst.tile([P, P], FP8)
            nc.sync.dma_start(out=id8t[:], in_=id8[:])
            ident = cst.tile([P, P], BF16)
            make_identity(nc, ident[:])

            regions = [(0, PS, tab8, FP8, id8t, 0),
                       (PS, npair, tab16, BF16, ident, int(pstart[PS]))]
            ci = 0
            for (plo, phi, tabsrc, tdt, rhsid, bb) in regions:
                for (p0, p1) in _chunks(kp, pstart, plo, phi):
                    cb0, cb1 = int(pstart[p0]) - bb, int(pstart[p1]) - bb
                    tabt = tp.tile([P, (cb1 - cb0) * P], tdt, tag="tabt")
                    hcol = (cb1 - cb0) * P // 2
                    eng0, eng1 = ((nc.sync, nc.scalar) if ci % 2 == 0
                                  else (nc.scalar, nc.sync))
                    ci += 1
                    eng0.dma_start(out=tabt[:, :hcol],
                                   in_=tabsrc[:, cb0 * P:cb0 * P + hcol])
                    eng1.dma_start(out=tabt[:, hcol:],
                                   in_=tabsrc[:, cb0 * P + hcol:cb1 * P])
                    outc = op.tile([P, (p1 - p0) * P], BF16, tag="outc")
                    for p in range(p0, p1):
                        boff = int(pstart[p]) - bb - cb0
                        acc = ps.tile([P, P], F32, space="PSUM", tag="acc")
                        for j in range(int(kp[p])):
                            nc.tensor.matmul(
                                out=acc[:],
                                lhsT=tabt[:, (boff + j) * P:(boff + j + 1) * P],
                                rhs=rhsid[:],
                                start=(j == 0), stop=False)
                        nc.tensor.matmul(out=acc[:], lhsT=ident[:],
                                         rhs=x2rpt[:, p * P:(p + 1) * P],
                                         start=False, stop=False)
                        nc.tensor.matmul(out=acc[:], lhsT=b2ct[:],
                                         rhs=onest[:],
                                         start=False, stop=True)
                        nc.scalar.copy(
                            out=outc[:, (p - p0) * P:(p - p0 + 1) * P],
                            in_=acc[:])
                    nc.gpsimd.dma_start(out=out[:, p0 * P:p1 * P], in_=outc[:])
    nc.compile()
    return nc


# ------------------------------------------------------------------- driver
def _run(nc, in_maps, trace=False):
    res = bass_utils.run_bass_kernel_spmd(
        nc, in_maps, core_ids=list(range(NCORES)), trace=trace)
    if res.exec_time_ns:
        _EXEC_NS.append(res.exec_time_ns)
    return res.results


def kernel(features, edges, edges2, edge_features,
           W1_l, b1_l, W1_r, W2_l, b2_l, W2_r, _trace=False):
    features = np.asarray(features, np.float32)
    src = np.asarray(edges[0], np.int64)
    dst = np.asarray(edges[1], np.int64)
    _EXEC_NS.clear()

    # ---- host prep
    (k_w, bstart, B, B8, wsplit, tabidx_all, tabscale_all,
     perms, pairs) = _prep_edges(src, dst)

    w1cat = np.zeros((CIN_P, 2 * C_HID), np.float32)
    w1cat[:C_IN, :C_HID] = np.asarray(W1_l, np.float32)
    w1cat[:C_IN, C_HID:] = np.asarray(W1_r, np.float32)
    w1cat[C_IN, C_HID:] = np.asarray(b1_l, np.float32)  # constant-1 row
    w1cat = _bf16(w1cat)

    w2cat = _bf16(np.concatenate([np.asarray(W2_l, np.float32),
                                  np.asarray(W2_r, np.float32)], axis=1))
    b2cat = _bf16(np.tile(np.asarray(b2_l, np.float32), 2).reshape(1, P))

    nsup = R // RSUP
    xts = []
    for m in range(NCORES):
        xt = np.zeros((CIN_P, R), ml_dtypes.bfloat16)
        perm = perms[m]
        cols = np.nonzero(perm >= 0)[0]
        xt[:C_IN, cols] = features[m * SHARD + perm[cols]].T
        xt[C_IN, cols] = 1.0
        xt3 = (xt.reshape(CT, KT, nsup, RSUP).transpose(2, 1, 0, 3)
               .reshape(nsup * KT, CT * RSUP))
        xts.append(np.ascontiguousarray(xt3))

    # ---- L1: both projections, channel-major
    nc1 = _build_l1()
    res1 = _run(nc1, [dict(xt=xts[m], w1=w1cat) for m in range(NCORES)],
                trace=_trace)
    hcat = np.concatenate(
        [np.asarray(res1[m]["ht_o"]).T for m in range(NCORES)],
        axis=0).astype(np.float32)

    # ---- L2: layer-1 aggregation + relu + layer-2 projections
    rows_all = [_tab_rows(hcat, tabidx_all[m], tabscale_all[m])
                for m in range(NCORES)]
    S2 = _pick_scale(rows_all, B8 * P)
    id8m = np.ascontiguousarray(
        (np.eye(P, dtype=np.float32) / S2).astype(E3M4))
    nc2 = _build_l2(k_w, bstart, B, B8, wsplit)
    in2 = []
    for m in range(NCORES):
        t8, t16 = _build_tab(rows_all[m], B8 * P, S2, C_HID)
        in2.append(dict(tab8=t8, tab16=t16, id8=id8m,
                        xrt=res1[m]["xrt_o"], w2=w2cat))
    res2 = _run(nc2, in2, trace=_trace)
    h2cat = np.concatenate(
        [np.asarray(res2[m]["o2"])[:C_OUT].T for m in range(NCORES)],
        axis=0).astype(np.float32)

    # ---- L3: layer-2 aggregation + output (paired windows)
    npair = NWIN // 2
    rows3p_all = [_pair_rows(_tab_rows(h2cat, tabidx_all[m],
                                       tabscale_all[m]), pairs)
                  for m in range(NCORES)]
    B8P = int(pairs["pstart"][pairs["PS"]])
    S3 = _pick_scale(rows3p_all, B8P * P)
    id83 = np.ascontiguousarray(
        (np.eye(P, dtype=np.float32) / S3).astype(E3M4))
    nc3 = _build_l3(pairs)
    in3 = []
    for m in range(NCORES):
        t8, t16 = _build_tab(rows3p_all[m], B8P * P, S3, P)
        x2r = np.asarray(res2[m]["o2"])[C_OUT:].reshape(C_OUT, NWIN, P)
        x2rp = np.empty((P, npair * P), ml_dtypes.bfloat16)
        x2rp[:C_OUT] = x2r[:, 0::2].reshape(C_OUT, npair * P)
        x2rp[C_OUT:] = x2r[:, 1::2].reshape(C_OUT, npair * P)
        in3.append(dict(tab8=t8, tab16=t16, id8=id83,
                        x2rp=np.ascontiguousarray(x2rp), b2c=b2cat))
    res3 = _run(nc3, in3, trace=_trace)

    out = np.empty((N_NODES, C_OUT), np.float32)
    for m in range(NCORES):
        o = np.asarray(res3[m]["out"]).astype(np.float32)
        o = o.reshape(2, C_OUT, npair, P)
        rowsR = np.empty((NWIN, P, C_OUT), np.float32)
        rowsR[0::2] = o[0].transpose(1, 2, 0)
        rowsR[1::2] = o[1].transpose(1, 2, 0)
        rowsR = rowsR.reshape(R, C_OUT)
        perm = perms[m]
        pos = np.nonzero(perm >= 0)[0]
        out[m * SHARD + perm[pos]] = rowsR[pos]
    return np.ascontiguousarray(out)
